# revision 10
# baseline (speedup 1.0000x reference)
"""Llama GQA attention layer (T=2048, H=4096, 32 q heads / 8 kv heads, hd=128),
tensor-parallel over heads across 8 Trainium2 NeuronCores.

Per core c: 4 q heads + 1 kv head (wq/wk/wv column slices, wo row slice).
Each core computes a full [T, H] partial o_proj output; partials are summed on
host (the all-reduce of the TP scheme).

Device layout trick: host feeds hiddenT [H, T] so every matmul contracts over
the partition dim. Attention scores are produced transposed (k on partitions),
so softmax normalization uses a ones-vector matmul for the denominator and the
probabilities feed the PV matmul directly as the moving operand. All matmul
operands are float32r (fp32 storage, ~1e-4 matmul precision, 4x the fp32 PE
throughput at N>=256).
"""

import sys

if "/opt/trn_rl_repo" not in sys.path:
    sys.path.insert(0, "/opt/trn_rl_repo")

import numpy as np

import concourse.bass as bass
import concourse.bacc as bacc
import concourse.tile as tile
import concourse.mybir as mybir
from concourse import bass_utils

T = 2048
H = 4096
NQ = 32
NKV = 8
HD = 128
THETA = 10000.0
N_CORES = 8
NH = NQ // N_CORES          # local q heads per core
HALF = HD // 2
TC = 512                    # t-chunk (matmul free dim)
NTC = T // TC               # 4
NKCH = H // 128             # 32 hidden chunks
SCALE = float(HD) ** -0.5

F32 = mybir.dt.float32
F32R = mybir.dt.float32r
ALU = mybir.AluOpType
ACTF = mybir.ActivationFunctionType


def _build():
    nc = bacc.Bacc("TRN2", target_bir_lowering=False, debug=False,
                   num_devices=N_CORES)
    ht = nc.dram_tensor("ht", [H, T], F32R, kind="ExternalInput").ap()
    wq = nc.dram_tensor("wq", [H, NH * HD], F32R, kind="ExternalInput").ap()
    wk = nc.dram_tensor("wk", [H, HD], F32R, kind="ExternalInput").ap()
    wv = nc.dram_tensor("wv", [H, HD], F32R, kind="ExternalInput").ap()
    wo = nc.dram_tensor("wo", [NH * HD, H], F32R, kind="ExternalInput").ap()
    cos2 = nc.dram_tensor("cos2", [HD, T], F32, kind="ExternalInput").ap()
    sin2 = nc.dram_tensor("sin2", [HD, T], F32, kind="ExternalInput").ap()
    ident = nc.dram_tensor("ident", [128, 128], F32R, kind="ExternalInput").ap()
    ones = nc.dram_tensor("ones", [128, 1], F32R, kind="ExternalInput").ap()
    out_t = nc.dram_tensor("out_t", [H, T], F32, kind="ExternalOutput").ap()

    with tile.TileContext(nc) as tc:
        _body(tc, ht, wq, wk, wv, wo, cos2, sin2, ident, ones, out_t)
    nc.compile()
    return nc


def _body(tc, ht, wq, wk, wv, wo, cos2, sin2, ident, ones, out_t):
    nc = tc.nc

    with (
        tc.tile_pool(name="persist", bufs=1) as persist,
        tc.tile_pool(name="small", bufs=1) as small,
    ):
        # live across all phases
        qT = persist.tile([128, NH * T], F32R, tag="qT")        # [d, h*T + t]
        kT = persist.tile([128, T], F32R, tag="kT")             # [d, t]
        v_sb = persist.tile([128, T], F32R, tag="v")            # chunk i cols: v[t=i*128+p, d]
        ident_sb = small.tile([128, 128], F32R, tag="ident")
        ones_sb = small.tile([128, 1], F32R, tag="ones")
        nc.sync.dma_start(ident_sb[:], ident[:, :])
        nc.sync.dma_start(ones_sb[:], ones[:, :])

        # ---------------- phase 1: QKV projections + RoPE + V transpose ----
        with (
            tc.tile_pool(name="ph1w", bufs=1) as ph1w,
            tc.tile_pool(name="ph1", bufs=4) as ph1,
            tc.tile_pool(name="rope", bufs=2) as rope,
            tc.tile_pool(name="ps1", bufs=1, space="PSUM") as ps1,
            tc.tile_pool(name="pst", bufs=2, space="PSUM") as pst,
        ):
            wq_sb = ph1w.tile([128, NKCH * (NH * HD)], F32R, tag="wq")
            wk_sb = ph1w.tile([128, NKCH * HD], F32R, tag="wk")
            wv_sb = ph1w.tile([128, NKCH * HD], F32R, tag="wv")
            cos_sb = ph1w.tile([128, T], F32, tag="cos")
            sin_sb = ph1w.tile([128, T], F32, tag="sin")
            vT_sb = ph1w.tile([128, T], F32R, tag="vT")
            nc.sync.dma_start(cos_sb[:], cos2[:, :])
            nc.sync.dma_start(sin_sb[:], sin2[:, :])
            for k in range(NKCH):
                nc.sync.dma_start(wq_sb[:, k * 512:(k + 1) * 512],
                                  wq[k * 128:(k + 1) * 128, :])
                nc.sync.dma_start(wk_sb[:, k * 128:(k + 1) * 128],
                                  wk[k * 128:(k + 1) * 128, :])
                nc.sync.dma_start(wv_sb[:, k * 128:(k + 1) * 128],
                                  wv[k * 128:(k + 1) * 128, :])

            for t in range(NTC):
                qps = [ps1.tile([128, TC], F32, tag=f"qps{fc}", name=f"qps{fc}")
                       for fc in range(NH)]
                kps = ps1.tile([128, TC], F32, tag="kps")
                vps = ps1.tile([128, TC], F32, tag="vps")
                for k in range(NKCH):
                    htt = ph1.tile([128, TC], F32R, tag="ht")
                    nc.sync.dma_start(htt[:], ht[k * 128:(k + 1) * 128,
                                                 t * TC:(t + 1) * TC])
                    st, sp = (k == 0), (k == NKCH - 1)
                    for fc in range(NH):
                        nc.tensor.matmul(
                            qps[fc][:],
                            wq_sb[:, k * 512 + fc * 128: k * 512 + (fc + 1) * 128],
                            htt[:], start=st, stop=sp)
                    nc.tensor.matmul(kps[:], wk_sb[:, k * 128:(k + 1) * 128],
                                     htt[:], start=st, stop=sp)
                    nc.tensor.matmul(vps[:], wv_sb[:, k * 128:(k + 1) * 128],
                                     htt[:], start=st, stop=sp)

                # RoPE: out = x*cos2 + swap(x)*sin2   (swap = halves exchanged)
                for hc in range(NH + 1):
                    src = qps[hc] if hc < NH else kps
                    dst = (qT[:, hc * T + t * TC: hc * T + (t + 1) * TC]
                           if hc < NH else kT[:, t * TC:(t + 1) * TC])
                    raw = rope.tile([128, TC], F32, tag="raw")
                    nc.scalar.copy(raw[:], src[:])
                    sw = rope.tile([128, TC], F32, tag="sw")
                    nc.sync.dma_start(sw[0:HALF, :], raw[HALF:128, :])
                    nc.sync.dma_start(sw[HALF:128, :], raw[0:HALF, :])
                    a = rope.tile([128, TC], F32, tag="ra")
                    b = rope.tile([128, TC], F32, tag="rb")
                    nc.vector.tensor_tensor(
                        a[:], raw[:], cos_sb[:, t * TC:(t + 1) * TC], ALU.mult)
                    nc.vector.tensor_tensor(
                        b[:], sw[:], sin_sb[:, t * TC:(t + 1) * TC], ALU.mult)
                    nc.vector.tensor_tensor(dst, a[:], b[:], ALU.add)
                # V has no rope; stash vT, transpose below
                nc.scalar.copy(vT_sb[:, t * TC:(t + 1) * TC], vps[:])

            # transpose vT -> v (natural [t, d] layout, chunked along free dim)
            for i in range(T // 128):
                tp = pst.tile([128, 128], F32R, tag="tp")
                nc.tensor.transpose(tp[:], vT_sb[:, i * 128:(i + 1) * 128],
                                    ident_sb[:])
                nc.scalar.copy(v_sb[:, i * 128:(i + 1) * 128], tp[:])

        # ---------------- phase 2+3: attention, then o_proj -----------------
        with (
            tc.tile_pool(name="late", bufs=1) as late,
            tc.tile_pool(name="att", bufs=3) as att,
            tc.tile_pool(name="rbp", bufs=2) as rbp,
            tc.tile_pool(name="ps_s", bufs=2, space="PSUM") as ps_s,
            tc.tile_pool(name="ps_o", bufs=2, space="PSUM") as ps_o,
            tc.tile_pool(name="ps_d", bufs=2, space="PSUM") as ps_d,
            tc.tile_pool(name="ps_op", bufs=2, space="PSUM") as ps_op,
        ):
            attnT = late.tile([128, NH * T], F32R, tag="attnT")   # [d, h*T + t]
            wo_sb = late.tile([128, NH * H], F32R, tag="wo")      # [f, h*H + m]
            for h in range(NH):
                nc.sync.dma_start(wo_sb[:, h * H:(h + 1) * H],
                                  wo[h * 128:(h + 1) * 128, :])

            for qc in range(NTC):
                nkc = (qc + 1) * (TC // 128)     # causal k chunks of 128
                for h in range(NH):
                    qh = qT[:, h * T + qc * TC: h * T + (qc + 1) * TC]
                    po = ps_o.tile([128, TC], F32, tag="po")
                    pd = ps_d.tile([1, TC], F32, tag="pd")
                    for kc in range(nkc):
                        sT = ps_s.tile([128, TC], F32, tag="sT")
                        nc.tensor.matmul(sT[:], kT[:, kc * 128:(kc + 1) * 128],
                                         qh, start=True, stop=True)
                        p = att.tile([128, TC], F32R, tag="p")
                        nc.scalar.activation(p[:], sT[:], ACTF.Exp, scale=SCALE)
                        di = kc - (nkc - 4)
                        if di >= 0:
                            # keep where dq - dk - 128*di >= 0, else 0
                            nc.gpsimd.affine_select(
                                p[:], p[:], pattern=[[1, TC]],
                                compare_op=ALU.is_ge, fill=0.0,
                                base=-128 * di, channel_multiplier=-1)
                        st, sp = (kc == 0), (kc == nkc - 1)
                        nc.tensor.matmul(po[:], v_sb[:, kc * 128:(kc + 1) * 128],
                                         p[:], start=st, stop=sp)
                        nc.tensor.matmul(pd[:], ones_sb[:], p[:],
                                         start=st, stop=sp)
                    rc = rbp.tile([1, TC], F32, tag="rc")
                    nc.vector.reciprocal(rc[:], pd[:])
                    rb = rbp.tile([128, TC], F32, tag="rbb")
                    nc.sync.dma_start(
                        rb[:], bass.AP(rc.tensor, rc.offset,
                                       [[1, 1], [0, 128], [1, TC]]))
                    nc.vector.tensor_tensor(
                        attnT[:, h * T + qc * TC: h * T + (qc + 1) * TC],
                        po[:], rb[:], ALU.mult)

            # o_proj: out_t[mo, t] += wo[f, mo] * attnT[f, t]
            for t in range(NTC):
                for mo in range(H // 128):
                    op = ps_op.tile([128, TC], F32, tag="op")
                    for h in range(NH):
                        nc.tensor.matmul(
                            op[:],
                            wo_sb[:, h * H + mo * 128: h * H + (mo + 1) * 128],
                            attnT[:, h * T + t * TC: h * T + (t + 1) * TC],
                            start=(h == 0), stop=(h == NH - 1))
                    ob = att.tile([128, TC], F32, tag="ob")
                    nc.vector.tensor_copy(ob[:], op[:])
                    nc.sync.dma_start(out_t[mo * 128:(mo + 1) * 128,
                                            t * TC:(t + 1) * TC], ob[:])


_NC = None
LAST_EXEC_NS = None
LAST_TRACE = None


def _ensure_profile_hook():
    """Register the axon NTFF profiling hook (container lacks antenv.axon_hooks)."""
    import types
    import antenv
    if "antenv.axon_hooks" in sys.modules:
        return
    hooks_mod = types.ModuleType("antenv.axon_hooks")
    _h = [None]
    hooks_mod.set_axon_ntff_profile_hook = lambda hk: _h.__setitem__(0, hk)
    hooks_mod.get_axon_ntff_profile_hook = lambda: _h[0]
    sys.modules["antenv.axon_hooks"] = hooks_mod
    antenv.axon_hooks = hooks_mod
    from trn_agent_boot.trn_boot import _ntff_profile_via_ctypes
    hooks_mod.set_axon_ntff_profile_hook(
        _ntff_profile_via_ctypes("/opt/axon/libaxon_pjrt.so"))
    bass_utils.upload_artifacts = lambda tmpdir: "local://skipped"


def kernel(positions, hidden_states, wq, wk, wv, wo, _trace=False, **_unused):
    global _NC, LAST_EXEC_NS, LAST_TRACE
    positions = np.asarray(positions)
    hidden_states = np.asarray(hidden_states, dtype=np.float32)
    wq = np.asarray(wq, dtype=np.float32)
    wk = np.asarray(wk, dtype=np.float32)
    wv = np.asarray(wv, dtype=np.float32)
    wo = np.asarray(wo, dtype=np.float32)

    # host-side input prep (sharding + layout)
    ht = np.ascontiguousarray(hidden_states.T)                      # [H, T]
    inv_freq = (1.0 / (THETA ** (np.arange(HALF, dtype=np.float64) / HALF)))
    ang = positions.astype(np.float64)[:, None] * inv_freq[None, :]  # [T, 64]
    cos = np.cos(ang).astype(np.float32).T                           # [64, T]
    sin = np.sin(ang).astype(np.float32).T
    cos2 = np.ascontiguousarray(np.concatenate([cos, cos], axis=0))  # [128, T]
    sin2 = np.ascontiguousarray(np.concatenate([-sin, sin], axis=0))
    ident = np.eye(128, dtype=np.float32)

    in_maps = []
    for c in range(N_CORES):
        in_maps.append({
            "ht": ht,
            "wq": np.ascontiguousarray(wq[:, c * NH * HD:(c + 1) * NH * HD]),
            "wk": np.ascontiguousarray(wk[:, c * HD:(c + 1) * HD]),
            "wv": np.ascontiguousarray(wv[:, c * HD:(c + 1) * HD]),
            "wo": np.ascontiguousarray(wo[c * NH * HD:(c + 1) * NH * HD, :]),
            "cos2": cos2,
            "sin2": sin2,
            "ident": ident,
            "ones": np.ones((128, 1), dtype=np.float32),
        })

    if _NC is None:
        _NC = _build()
    if _trace:
        _ensure_profile_hook()
    res = bass_utils.run_bass_kernel_spmd(
        _NC, in_maps, core_ids=list(range(N_CORES)), trace=_trace)
    if _trace:
        LAST_EXEC_NS = res.exec_time_ns
        LAST_TRACE = (res.instructions_and_trace[1]
                      if res.instructions_and_trace else None)

    acc = res.results[0]["out_t"].astype(np.float64)
    for c in range(1, N_CORES):
        acc += res.results[c]["out_t"]
    return np.ascontiguousarray(acc.T).astype(np.float32)
